# revision 57
# baseline (speedup 1.0000x reference)
"""MultiHeadChannelAttention Bass kernel for 8 Trainium2 NeuronCores.

Problem (hardcoded shapes): x (2, 512, 64, 32) fp32; Wq/Wk/Wv/Wfc (512, 512);
biases (512,). Reference math per batch b, with X = x[b].reshape(2048, 512):
  Q = X Wq^T + bq ; K = X Wk^T + bk ; V = X Wv^T + bv   (heads of 64 dims)
  out = softmax(QK^T/8) V  (per head), concat heads, @ Wfc^T + bfc

Sharding: 8 cores = 2 batches x 4 token-blocks of 512 tokens. Each core
computes K/V for all 2048 tokens of its batch (4x redundant), Q/attention/fc
only for its 512-token block. No cross-core communication.

On this hardware a DoubleRow fp8 matmul streams 1 col/cycle like bf16 but
contracts 2 rows per partition, so fp8 pays off exactly where it doubles
the contraction per pass:
  - Q/K projections: x and Wq/Wk pre-cast to fp8 (weights scaled by S=32 on
    the host to clear e4m3's subnormal range; Wfc absorbs 1/S), 256-channel
    contraction per pass -- 2 passes instead of bf16's 4.
  - attnV: two key-tiles (128 keys each) contract per pass; exp writes fp8
    directly; V carries a ones column so the softmax denominator falls out
    of the same matmul (V itself is projected in bf16 for accuracy and cast
    to fp8 on the PSUM->SBUF copy).
Scores stay at the 64-wide-contraction layout straight off the fp8
projection outputs (the PE overlaps the paired stationary loads, which
beats the 32x2 DoubleRow restructure measured on hardware).

ScalarE (ACT) runs only the 64 [128,1024] exps -- it is the pacing engine
(~69 us busy). Softmax reciprocals run on DVE; the denominator broadcast
over dk is a PE ones-matmul. bv is folded into the fc bias on host
(softmax rows sum to 1). The fc tail stays bf16; output ships as bf16.
"""

import numpy as np
import ml_dtypes

N_CORES = 8
B, C, N_TOK, TB = 2, 512, 2048, 512
HEADS, DK = 8, 64
NCH = C // 128  # channel chunks (4)
NPC = NCH // 2  # chunk pairs for fp8 DoubleRow projections (2)
NJT = N_TOK // 128  # key-token tiles (16)
NT = NJT // 2  # key-tile pairs per attnV pass (8)
NTT = TB // 128  # fc token tiles (4)
WSCALE = 32.0  # host pre-scale keeping fp8 weights out of subnormals

_CACHE = {}


def _install_tile_drain_patch():
    """The end-of-kernel Tile drain can carry several sem waits; this
    walrus build allows one wait per non-EVSEM instruction. Split the
    waits across a chain of drains."""
    import bass_rust
    from concourse import tile as _tile
    from concourse.vector_clock import ScopedClock

    if getattr(_tile.TileContext, "_drain_patch_installed", False):
        return

    def _patched(self, tick_clock, wait_clock):
        nc = self.nc
        drain_inst = nc.sync.drain()
        wait_clock.add_sem_waits(
            drain_inst.ins, ScopedClock({None: tick_clock.global_clock})
        )
        si = drain_inst.ins.sync_info
        if si is not None and len(si.on_wait) > 1:
            waits = list(si.on_wait)
            drain_inst.ins.sync_info = bass_rust.SyncInfo(
                on_wait=[waits[0]], on_update=list(si.on_update)
            )
            for w in waits[1:]:
                extra = nc.sync.drain()
                extra.ins.sync_info = bass_rust.SyncInfo(on_wait=[w], on_update=[])
        nc.all_engine_barrier()
        assert self.sems is not None
        popped = nc._tile_sem_poison_stack.pop()
        assert popped is self._sem_poison
        nc.clear_and_free_semaphores(list(self.sems.allocated().values()))
        nc.all_engine_barrier()

    _tile.TileContext._drain_and_barrier = _patched
    _tile.TileContext._drain_patch_installed = True


def _split_multi_waits(nc):
    """This walrus build accepts one sync wait per instruction (two on
    EVSEM). Tile can attach two; move extras onto preceding NOPs."""
    import concourse.mybir as mybir

    for f in nc.m.functions:
        for bb in f.blocks:
            out = []
            changed = False
            for ins in bb.instructions:
                si = ins.sync_info
                limit = 2 if isinstance(ins, mybir.InstEventSemaphore) else 1
                if si is not None and len(si.on_wait) > limit:
                    waits = list(si.on_wait)
                    keep = waits[-limit:]
                    for i, w in enumerate(waits[:-limit]):
                        nop = mybir.InstNoOp(
                            name=f"{ins.name}_w{i}",
                            engine=ins.engine,
                            sync_info=mybir.SyncInfo(on_wait=[w], on_update=[]),
                            bass_nofuse=True,
                        )
                        nc.register_instruction(nop, overwrite=True)
                        out.append(nop)
                    ins.sync_info = mybir.SyncInfo(
                        on_wait=keep, on_update=list(si.on_update)
                    )
                    changed = True
                out.append(ins)
            if changed:
                bb.instructions = out


def _build():
    import concourse.bass as bass
    import concourse.mybir as mybir
    import concourse.tile as tile
    from concourse.bass import ts

    dt = mybir.dt
    f32, bf16, f8 = dt.float32, dt.bfloat16, dt.float8e4
    Exp = mybir.ActivationFunctionType.Exp
    Ln = mybir.ActivationFunctionType.Ln
    Copy = mybir.ActivationFunctionType.Copy
    DR = mybir.MatmulPerfMode.DoubleRow
    EXP_SCALE = 0.125 / (WSCALE * WSCALE)

    nc = bass.Bass()
    # fp8 Q/K operands, host pre-interleaved: chunk-pair axes are (pc, i)
    # with input channel c = 128*(2*pc+i)+partition
    xf8_d = nc.dram_tensor("xf8", [128, NPC, 2, N_TOK], f8, kind="ExternalInput")
    xq8_d = nc.dram_tensor("xq8", [128, NPC, 2, TB], f8, kind="ExternalInput")
    wq2_d = nc.dram_tensor("wq2", [128, NPC, 2, C], f8, kind="ExternalInput")
    wk2_d = nc.dram_tensor("wk2", [128, NPC, 2, C], f8, kind="ExternalInput")
    wv2_d = nc.dram_tensor("wv2", [128, NPC, 2, C], f8, kind="ExternalInput")
    wfT_d = nc.dram_tensor("wfT", [128, NCH * C], bf16, kind="ExternalInput")
    bias_d = nc.dram_tensor("bias", [128, 2 * NCH], f32, kind="ExternalInput")
    bfc_d = nc.dram_tensor("bfc", [1, C], bf16, kind="ExternalInput")
    out_d = nc.dram_tensor("out", [TB, C], bf16, kind="ExternalOutput")

    with tile.TileContext(nc) as tc:
        with (
            tc.tile_pool(name="wp", bufs=1) as wp,
            tc.tile_pool(name="data", bufs=1) as data,
            tc.tile_pool(name="ep", bufs=4) as ep,
            tc.tile_pool(name="np_", bufs=2) as npool,
            tc.tile_pool(name="scp", bufs=2, space=bass.MemorySpace.PSUM) as scp,
            tc.tile_pool(name="ap_", bufs=1, space=bass.MemorySpace.PSUM) as apool,
            tc.tile_pool(name="aux", bufs=2, space=bass.MemorySpace.PSUM) as aux,
        ):
            # ---- constants / weights ----
            wq2 = wp.tile([128, NPC, 2, C], f8, tag="wq", name="wq2")
            wk2 = wp.tile([128, NPC, 2, C], f8, tag="wk", name="wk2")
            wv2 = wp.tile([128, NPC, 2, C], f8, tag="wv", name="wv2")
            wf_all = wp.tile([128, NCH * C], bf16, tag="wf", name="wf_all")
            wf = [wf_all[:, ts(c, C)] for c in range(NCH)]
            bias_all = wp.tile([128, 2 * NCH], f32, tag="bias", name="bias_all")
            bqt = [bias_all[:, d : d + 1] for d in range(NCH)]
            bkt = [bias_all[:, NCH + d : NCH + d + 1] for d in range(NCH)]
            bfct = wp.tile([1, C], bf16, tag="bfct", name="bfct")
            ones_t = wp.tile([128, TB], bf16, tag="ones", name="ones_t")
            nc.vector.memset(ones_t[:], 1.0)
            ones_f = wp.tile([128, 64], f32, tag="onesf", name="ones_f")
            nc.vector.memset(ones_f[:], 1.0)

            # preload the Exp ACT table during the input-DMA window
            actwarm = npool.tile([1, 8], f32, tag="actw", name="actwarm")
            nc.scalar.activation(out=actwarm[:], in_=ones_f[0:1, 0:8], func=Exp)

            # PE warmup: dummy matmuls ramp the HAM activity monitor through
            # the input-load window so the projections run at speed
            for g in range(3):
                warm = aux.tile([128, TB], f32, tag="aux", name=f"warm{g}")
                for r in range(8):
                    nc.tensor.matmul(
                        warm[:], ones_t[0:1, 0:128], ones_t[0:1, :],
                        start=(r == 0), stop=(r == 7),
                    )

            # ---- activations in ----
            xf8 = data.tile([128, NPC, 2, N_TOK], f8, tag="xf8", name="xf8")
            xq8 = data.tile([128, NPC, 2, TB], f8, tag="xq8", name="xq8")

            # ---- input DMAs. Time-to-first-exp is bound by bias+wk2+
            # xf8(jb0)+xq8+wq2; xt/wv (V path) are due by pair0 t=2 ----
            nc.sync.dma_start(out=bias_all[:], in_=bias_d[:])
            nc.scalar.dma_start(out=wq2[:], in_=wq2_d[:])
            nc.sync.dma_start(out=wk2[:], in_=wk2_d[:])
            nc.scalar.dma_start(out=xq8[:], in_=xq8_d[:])
            nc.sync.dma_start(out=xf8[:, :, :, ts(0, TB)], in_=xf8_d[:, :, :, ts(0, TB)])
            nc.gpsimd.dma_start(out=wv2[:], in_=wv2_d[:])
            nc.sync.dma_start(
                out=xf8[:, :, :, TB:N_TOK], in_=xf8_d[:, :, :, TB:N_TOK]
            )
            nc.gpsimd.dma_start(out=wf_all[:], in_=wfT_d[:])
            nc.gpsimd.dma_start(out=bfct[:], in_=bfc_d[:])

            # ---- persistent intermediates ----
            # K^T/Q^T stay bf16: scores then run the proven head-paired
            # 64-contraction matmuls (disjoint partition halves -> the PE
            # overlaps the second stationary load almost entirely)
            kt = [
                data.tile([128, N_TOK], bf16, tag=f"kt{d}", name=f"kt{d}")
                for d in range(NCH)
            ]
            qt = [
                data.tile([128, TB], bf16, tag=f"qt{d}", name=f"qt{d}")
                for d in range(NCH)
            ]
            # V pairs: [128p tok, half i, head, dk+ones+pad] -- the pad
            # column keeps the DoubleRow Ldweights length even
            vpad2 = [
                data.tile([128, 2, HEADS, DK + 2], f8, tag=f"vp{t}", name=f"vp{t}")
                for t in range(NT)
            ]
            # exp outputs, regrouped per key-tile pair: [128p j, half i,
            # head hh, query] -- the exp writes a contiguous [128,1024]
            # half; attnV's DoubleRow rhs strides over the hh slice
            ebuf = [
                ep.tile([128, 2, 2, TB], f8, tag="e", name=f"eb{i}")[:]
                for i in range(4)
            ]
            att = [
                data.tile([128, TB], bf16, tag=f"att{c}", name=f"att{c}")
                for c in range(NCH)
            ]
            for t in range(NT):
                nc.vector.memset(vpad2[t][:, :, :, DK : DK + 2], 0.0)
                nc.vector.memset(vpad2[t][:, :, :, DK : DK + 1], 1.0)

            def proj_q(d):
                """Q^T d-tile (128 chans = heads 2d, 2d+1) + bias -> bf16.
                fp8 DoubleRow contracts 256 channels per pass."""
                qp = aux.tile([128, TB], f32, tag="aux", name=f"qp{d}")
                for pc in range(NPC):
                    nc.tensor.matmul(
                        qp[:], wq2[:, pc, :, ts(d, 128)], xq8[:, pc],
                        start=(pc == 0), stop=(pc == NPC - 1), perf_mode=DR,
                    )
                nc.vector.tensor_scalar_add(out=qt[d][:], in0=qp[:], scalar1=bqt[d])

            def proj_k(d, jb):
                """K^T d-tile, token block jb + bias -> bf16."""
                kp = aux.tile([128, TB], f32, tag="aux", name=f"kp{d}_{jb}")
                for pc in range(NPC):
                    nc.tensor.matmul(
                        kp[:], wk2[:, pc, :, ts(d, 128)], xf8[:, pc, :, ts(jb, TB)],
                        start=(pc == 0), stop=(pc == NPC - 1), perf_mode=DR,
                    )
                nc.vector.tensor_scalar_add(
                    out=kt[d][:, ts(jb, TB)], in0=kp[:], scalar1=bkt[d]
                )

            def proj_v(j):
                """V j-tile (fp8 DoubleRow, 256-chan contraction per pass)
                -> vpad2[j//2] half j%2, fp8."""
                vp = aux.tile([128, C], f32, tag="aux", name=f"vpp{j}")
                for pc in range(NPC):
                    nc.tensor.matmul(
                        vp[:], xf8[:, pc, :, ts(j, 128)], wv2[:, pc],
                        start=(pc == 0), stop=(pc == NPC - 1), perf_mode=DR,
                    )
                nc.vector.tensor_copy(
                    out=vpad2[j // 2][:, j % 2, :, 0:DK],
                    in_=vp[:].rearrange("p (h d) -> p h d", h=HEADS),
                )

            def scores_exp(p, j):
                """Scores for j-tile j, both heads of pair p (head-paired
                matmuls on disjoint partition halves), then one exp into the
                strided half of ebuf[(j//2) % 4]."""
                sc = scp.tile([128, 2, TB], f32, tag="sc", name=f"sc{p}_{j}")
                nc.tensor.matmul(
                    sc[:, 0, :], kt[p][0:64, ts(j, 128)], qt[p][0:64, :]
                )
                nc.tensor.matmul(
                    sc[:, 1, :], kt[p][64:128, ts(j, 128)], qt[p][64:128, :]
                )
                e = ebuf[(j // 2) % 4]
                nc.scalar.activation(
                    out=e[:, j % 2], in_=sc[:], func=Exp, scale=EXP_SCALE
                )

            def attn_v(p, t, hh, a):
                nc.tensor.matmul(
                    a[:], vpad2[t][:, :, 2 * p + hh, :], ebuf[t % 4][:, :, hh, :],
                    start=(t == 0), stop=(t == NT - 1), perf_mode=DR,
                )

            def normalize(pp, a_sb, hh):
                """Softmax normalization for pair pp's head hh. Denominator
                reciprocal on DVE; broadcast over the 64 dk partitions via a
                PE ones-matmul; scale on DVE."""
                rb = aux.tile([64, TB], f32, tag="aux", name=f"rb{pp}_{hh}")
                rcp = npool.tile([65, TB], f32, tag="rcp", bufs=4, name=f"rcp{pp}_{hh}")
                nc.vector.reciprocal(out=rcp[64:65, :], in_=a_sb[64:65, :])
                nc.tensor.matmul(rb[:], ones_f[64:65, :], rcp[64:65, :])
                nc.vector.tensor_mul(
                    out=att[pp][ts(hh, 64), :], in0=a_sb[0:64, :], in1=rb[:]
                )

            def fc_prefill(tt, fp):
                nc.tensor.matmul(
                    fp, ones_t[0:1, 0:128], bfct[:], start=True, stop=False
                )
                for c in range(NCH - 1):
                    nc.tensor.matmul(
                        fp, att[c][:, ts(tt, 128)], wf[c],
                        start=False, stop=False,
                    )

            # ---- main pipeline ----
            fcs = {}
            proj_q(0)
            proj_k(0, 0)
            prev = None  # previous pair's SBUF accumulator copies
            pending = None  # previous pair's boundary work, run at (p, j=0)
            for p in range(NCH):  # head pair p = heads 2p, 2p+1
                a0 = apool.tile([DK + 2, TB], f32, tag="a0", name=f"a0_{p}")
                a1 = apool.tile([DK + 2, TB], f32, tag="a1", name=f"a1_{p}")
                for j in range(NJT):
                    # pair seam: emit a two-step scores runway before the
                    # previous pair's boundary leftovers, so the exp stream
                    # never waits on the PE's in-order queue
                    if not (p >= 1 and j == 1):
                        scores_exp(p, j)
                    if p >= 1 and j == 0:
                        scores_exp(p, 1)
                        prev = pending()
                        pending = None
                    # pair 0: remaining K blocks, paced with the xf8 DMA
                    if p == 0 and j in (1, 3, 5):
                        proj_k(0, (j + 1) // 2)
                    # V projections ride pair 0 one tile per step, one
                    # key-pair ahead of its own attnV consumption
                    if p == 0 and j >= 1:
                        proj_v(j - 1)
                    # next pair's K/Q projections, spread mid-pair
                    if p < NCH - 1:
                        if j == 8:
                            proj_q(p + 1)
                        elif j in (10, 12, 14):
                            proj_k(p + 1, (j - 10) // 2)
                    else:
                        # last pair: fc pre-accumulation (bias + chunks 0..2)
                        # as PE filler; fp0/fp1 in the freed aux slots
                        if j in (6, 8):
                            fcs[j // 2 - 3] = aux.tile(
                                [128, C], f32, tag="aux", name=f"fp{j // 2 - 3}"
                            )[:]
                            fc_prefill(j // 2 - 3, fcs[j // 2 - 3])
                    # previous pair's normalization, deferred into this pair
                    if prev is not None and j in (2, 4):
                        normalize(p - 1, prev[j // 2 - 1], j // 2 - 1)
                    # attnV per key-tile pair at odd j; pair 0 lags one pair
                    # so the xt/wv DMAs and V projections stay off the
                    # exp-stream critical path
                    if j % 2 == 1:
                        tv = (j - 3) // 2 if p == 0 else j // 2
                        if tv >= 0:
                            attn_v(p, tv, 0, a0)
                            attn_v(p, tv, 1, a1)

                def boundary(p=p, a0=a0, a1=a1):
                    if p == 0:
                        proj_v(NJT - 1)
                        attn_v(p, NT - 1, 0, a0)
                        attn_v(p, NT - 1, 1, a1)
                    proj_k(p + 1, 3)
                    # evacuate accumulators to SBUF (DVE) so the banks free
                    a_sb0 = npool.tile(
                        [DK + 1, TB], f32, tag="asb", bufs=4, name=f"asb0_{p}"
                    )
                    a_sb1 = npool.tile(
                        [DK + 1, TB], f32, tag="asb", bufs=4, name=f"asb1_{p}"
                    )
                    nc.vector.tensor_copy(out=a_sb0[:], in_=a0[0 : DK + 1, :])
                    nc.vector.tensor_copy(out=a_sb1[:], in_=a1[0 : DK + 1, :])
                    return (a_sb0, a_sb1)

                if p < NCH - 1:
                    pending = boundary
                else:
                    # last pair: normalize straight out of PSUM at the tail
                    prev = (a0, a1)

            # ---- tail: only the final fc chunk waits on normalize(3).
            # a0/a1 stay live (PSUM-direct reciprocal); head 6's reciprocal
            # runs on ACT (idle now; Ln/Exp/Copy share the loaded table),
            # head 7's on DVE, in parallel. The freed a0/a1 slots take the
            # rb broadcasts; fp2/fp3 take the scores slots ----
            a0, a1 = prev
            fp2 = scp.tile([128, 2, C], f32, tag="sc", name="fp2")
            fp3 = scp.tile([128, 2, C], f32, tag="sc", name="fp3")
            fcs[2], fcs[3] = fp2[:, 0, :], fp3[:, 0, :]
            fc_prefill(2, fcs[2])
            fc_prefill(3, fcs[3])

            lnt0 = npool.tile([65, TB], f32, tag="rcp", bufs=4, name="lnt3_0")
            rcp0 = npool.tile([65, TB], f32, tag="rcp", bufs=4, name="rcp3_0")
            rcp1 = npool.tile([65, TB], f32, tag="rcp", bufs=4, name="rcp3_1")
            asb0 = npool.tile([DK + 1, TB], f32, tag="asb", bufs=4, name="asb3_0")
            asb1 = npool.tile([DK + 1, TB], f32, tag="asb", bufs=4, name="asb3_1")
            nc.scalar.activation(out=lnt0[64:65, :], in_=a0[64:65, :], func=Ln)
            nc.vector.reciprocal(out=rcp1[64:65, :], in_=a1[64:65, :])
            nc.scalar.activation(
                out=rcp0[64:65, :], in_=lnt0[64:65, :], func=Exp, scale=-1.0
            )
            nc.scalar.activation(out=asb0[:], in_=a0[0 : DK + 1, :], func=Copy)
            nc.vector.tensor_copy(out=asb1[:], in_=a1[0 : DK + 1, :])
            rb0 = apool.tile([64, TB], f32, tag="a0", name="rb3_0")
            rb1 = apool.tile([64, TB], f32, tag="a1", name="rb3_1")
            nc.tensor.matmul(rb0[:], ones_f[64:65, :], rcp0[64:65, :])
            nc.vector.tensor_mul(
                out=att[3][0:64, :], in0=asb0[0:64, :], in1=rb0[:]
            )
            nc.tensor.matmul(rb1[:], ones_f[64:65, :], rcp1[64:65, :])
            nc.vector.tensor_mul(
                out=att[3][64:128, :], in0=asb1[0:64, :], in1=rb1[:]
            )

            for tt in range(NTT):
                nc.tensor.matmul(
                    fcs[tt], att[3][:, ts(tt, 128)], wf[3],
                    start=False, stop=True,
                )
                # evacuate on ACT (idle at the tail; Copy shares Exp's table)
                ot = npool.tile([128, C], bf16, tag="ot", bufs=4, name=f"ot{tt}")
                nc.scalar.activation(out=ot[:], in_=fcs[tt], func=Copy)
                (nc.sync if tt % 2 == 0 else nc.gpsimd).dma_start(
                    out=out_d[ts(tt, 128), :], in_=ot[:]
                )

    _split_multi_waits(nc)
    nc.finalize()
    return nc


def get_nc():
    if "nc" not in _CACHE:
        _install_tile_drain_patch()
        _CACHE["nc"] = _build()
    return _CACHE["nc"]


def make_in_maps(x, Wq, bq, Wk, bk, Wv, bv, Wfc, bfc):
    bf = ml_dtypes.bfloat16
    f8 = ml_dtypes.float8_e4m3
    x = np.asarray(x, np.float32)
    Wq, Wk, Wv, Wfc = (np.asarray(w, np.float32) for w in (Wq, Wk, Wv, Wfc))
    bq, bk, bv, bfc = (np.asarray(v, np.float32) for v in (bq, bk, bv, bfc))
    S = np.float32(WSCALE)

    def interleave(wT):
        # [C, cols] -> [128, NCH*cols] with chunk c at columns [c*cols:...]
        cols = wT.shape[1]
        return np.ascontiguousarray(
            wT.reshape(NCH, 128, cols).transpose(1, 0, 2).reshape(128, NCH * cols)
        )

    def dr_pack(m):
        # [C, cols] -> [128, NPC, 2, cols]; input channel 128*(2pc+i)+p
        cols = m.shape[1]
        return np.ascontiguousarray(m.reshape(NPC, 2, 128, cols).transpose(2, 0, 1, 3))

    bfc_folded = (Wfc @ bv + bfc).reshape(1, C).astype(bf)
    wq2 = dr_pack((np.ascontiguousarray(Wq.T) * S).astype(f8))
    wk2 = dr_pack((np.ascontiguousarray(Wk.T) * S).astype(f8))
    wv2 = dr_pack((np.ascontiguousarray(Wv.T) * S).astype(f8))
    wfT = interleave(np.ascontiguousarray((Wfc / S).T).astype(bf))
    bias_c = (
        np.concatenate([bq.reshape(NCH, 128).T, bk.reshape(NCH, 128).T], axis=1) * S
    ).astype(np.float32)

    in_maps = []
    for core in range(N_CORES):
        b, t = divmod(core, N_TOK // TB)
        XT = np.ascontiguousarray(x[b].reshape(N_TOK, C).T)
        in_maps.append(
            {
                "xf8": dr_pack(XT.astype(f8)),
                "xq8": dr_pack(
                    np.ascontiguousarray(XT[:, t * TB : (t + 1) * TB]).astype(f8)
                ),
                "wq2": wq2,
                "wk2": wk2,
                "wv2": wv2,
                "wfT": wfT,
                "bias": bias_c,
                "bfc": bfc_folded,
            }
        )
    return in_maps


def assemble(outs):
    """outs: list of 8 dicts with 'out' (512, 512) -> (2, 512, 64, 32)."""
    per_batch = [
        np.concatenate([outs[b * 4 + t]["out"] for t in range(4)], axis=0)
        for b in range(B)
    ]
    return np.stack(per_batch).reshape(B, C, 64, 32).astype(np.float32)


def kernel(**inputs):
    from concourse.bass_utils import run_bass_kernel_spmd

    nc = get_nc()
    in_maps = make_in_maps(**inputs)
    res = run_bass_kernel_spmd(nc, in_maps, list(range(N_CORES)))
    return assemble(res.results)
